# revision 13
# baseline (speedup 1.0000x reference)
"""Trainium2 Bass kernel for nn_Decoder_14680198217759.

Multi-head attention decoder (B=32, G=N=512, E=128, H=8, D=16), pure data
parallel over 8 NeuronCores (4 batches/core).

v3 design notes (cost-model driven):
  - Act engine is the floor: exp of 8 heads x 512x512 per batch must run on
    the Activation engine (~16 x [128,1024] exps/batch, ~22us/batch).
    Everything else is arranged to hide under it.
  - All activations/weights flow as fp16 (mask as bf16, holds -1e9): PE
    streams at 1 cyc/col, DVE copies get the 2x 16-bit mode, DMA bytes halve.
  - Activations are loaded natural via gpsimd casting DMA (f32->fp16), then
    transposed E-major on the PE (4 transposes fused into one [128,512] PSUM
    tile, one copy out).
  - Scores are computed transposed per head pair into [128,1024] PSUM; the
    rank-3 mask rides as the leading matmul of each accumulation group
    (lhsT=I, rhs=maskT chunk). exp on Act -> fp16 SBUF.
  - PV is FLIPPED: lhsT = exp^T chunk (stationary, free weight load), rhs =
    v_aug 17-col block (16 v cols + ones col for the denominator): 17-col
    output streams instead of 512. Output lands NATURAL [g, 17h] in PSUM, so
    per-head normalization is per-partition tensor_scalar ops, no transpose
    dance. (GPSIMD cannot read PSUM: u is staged to SBUF first.)
  - Branch 2 mask is added into the s2 PSUM before tanh: tanh saturates to
    -1 for masked entries, exp(10*-1)=e-10 ~ 0 (error ~1e-5 relative).
    Denominators come free via activation accum_out.
  - Branch 2 of batch b-1 is software-pipelined into the middle of batch b's
    attention so the Act engine never waits for the PV->normalize->combine
    chain.
  - Output stored fp16, host casts to f32.
"""

import numpy as np

B, G, N, E, H, D = 32, 512, 512, 128, 8, 16
SQRT_E = 11.313708498984761
NCORES = 8
BL = B // NCORES  # batches per core

WNAMES = ["Wq1p0", "Wq1p1", "Wqlp0", "Wqlp1", "Wkp0", "Wkp1", "Wv", "Wc"]

_CACHE = {}


# --------------------------------------------------------------------------
# BIR wait legalization: this toolchain's walrus accepts at most ONE sem wait
# per instruction; Tile's scheduler can emit more (notably on the kernel-tail
# drain). Split excess waits onto same-engine NoOps placed directly before
# the offending instruction (same-queue program order keeps the semantics).
# --------------------------------------------------------------------------
def _legalize_waits(nc, max_waits=1):
    import concourse.mybir as mybir

    n_split = 0
    for f in nc.m.functions:
        for bb in f.blocks:
            out = []
            for ins in bb.instructions:
                si = ins.sync_info
                waits = list(si.on_wait) if si and si.on_wait else []
                if len(waits) > max_waits:
                    while len(waits) > max_waits:
                        chunk, waits = waits[:max_waits], waits[max_waits:]
                        nop = mybir.InstNoOp(
                            name=f"I-waitfix-{nc.next_id()}", ins=[], outs=[]
                        )
                        nop.engine = ins.engine
                        nop.sync_info = mybir.SyncInfo(on_wait=chunk, on_update=[])
                        out.append(nop)
                        n_split += 1
                    ins.sync_info = mybir.SyncInfo(
                        on_wait=waits, on_update=list(si.on_update or [])
                    )
                out.append(ins)
            bb.instructions[:] = out
    return n_split


def _build_nc(legalize=True):
    import concourse.bass as bass
    import concourse.mybir as mybir
    import concourse.tile as tile
    from concourse.masks import make_identity

    f32 = mybir.dt.float32
    f16 = mybir.dt.float16
    bf16 = mybir.dt.bfloat16
    AF = mybir.ActivationFunctionType

    nc = bass.Bass()

    nodes_d = nc.dram_tensor("nodes", [BL, N, E], f32, kind="ExternalInput")
    q1_d = nc.dram_tensor("q1", [BL, G, E], f32, kind="ExternalInput")
    last_d = nc.dram_tensor("last", [BL, G, E], f32, kind="ExternalInput")
    mask_d = nc.dram_tensor("mask", [BL, G, N], f32, kind="ExternalInput")
    wcat_d = nc.dram_tensor("wcat", [E, 8, 128], f16, kind="ExternalInput")
    b_d = nc.dram_tensor("bc", [E, 1], f32, kind="ExternalInput")
    probs_d = nc.dram_tensor("probs", [BL, G, N], f16, kind="ExternalOutput")

    with tile.TileContext(nc) as tc:
        import contextlib

        with contextlib.ExitStack() as ctx:
            pw = ctx.enter_context(tc.tile_pool(name="pw", bufs=1))
            pin = ctx.enter_context(tc.tile_pool(name="pin", bufs=2))
            pxt = ctx.enter_context(tc.tile_pool(name="pxt", bufs=2))
            pproj = ctx.enter_context(tc.tile_pool(name="pproj", bufs=2))
            pexp = ctx.enter_context(tc.tile_pool(name="pexp", bufs=6))
            pmisc = ctx.enter_context(tc.tile_pool(name="pmisc", bufs=2))
            pstage = ctx.enter_context(tc.tile_pool(name="pstage", bufs=2))
            ps_score = ctx.enter_context(
                tc.tile_pool(name="ps_score", bufs=2, space="PSUM")
            )
            ps_u = ctx.enter_context(tc.tile_pool(name="ps_u", bufs=1, space="PSUM"))
            ps_m = ctx.enter_context(tc.tile_pool(name="ps_m", bufs=2, space="PSUM"))

            # ---- constants / weights (one batched DMA) ----
            ident = pw.tile([128, 128], f16)
            make_identity(nc, ident)
            ident_b = pw.tile([128, 128], bf16)
            make_identity(nc, ident_b)
            ident_f = pw.tile([128, 128], f32)
            make_identity(nc, ident_f)
            wcat = pw.tile([128, 8, 128], f16)
            nc.sync.dma_start(out=wcat, in_=wcat_d[:, :, :])
            w_sb = {n: wcat[:, i, :] for i, n in enumerate(WNAMES)}
            b_sb = pw.tile([128, 1], f32)
            nc.sync.dma_start(out=b_sb, in_=b_d[:, :])

            # state carried between loop iterations for the software-pipelined
            # branch 2 (runs one batch behind the attention stream)
            prev = {}

            def branch2(st):
                t_sb = pstage.tile([128, 4, 512], f16, tag="tsb", name="t_sb")
                for gp in range(2):
                    s2 = ps_score.tile([128, 1024], f32, tag="sc", name="s2")
                    for k2 in range(2):
                        gc = 2 * gp + k2
                        sl = s2[:, k2 * 512 : (k2 + 1) * 512]
                        nc.tensor.matmul(
                            sl,
                            st["mhT"][:, gc * 128 : (gc + 1) * 128],
                            st["nodesT"],
                            start=True,
                            stop=False,
                        )
                        nc.tensor.matmul(
                            sl,
                            ident_b,
                            st["mask_t"][:, gc, :],
                            start=False,
                            stop=True,
                        )
                    nc.scalar.activation(
                        t_sb[:, 2 * gp : 2 * gp + 2, :],
                        s2,
                        AF.Tanh,
                        scale=1.0 / SQRT_E,
                    )
                den = pmisc.tile([128, 4], f32, tag="den", name="den")
                p_sb = pstage.tile([128, 4, 512], f16, tag="psb", name="p_sb")
                for gc in range(4):
                    nc.scalar.activation(
                        p_sb[:, gc, :],
                        t_sb[:, gc, :],
                        AF.Exp,
                        scale=10.0,
                        accum_out=den[:, gc : gc + 1],
                    )
                recipden = pmisc.tile([128, 4], f32, tag="rd", name="recipden")
                nc.vector.reciprocal(recipden, den)
                stage = pstage.tile([128, 4, 512], f16, tag="stg", name="stage")
                for gc in range(4):
                    nc.vector.tensor_scalar_mul(
                        stage[:, gc, :], p_sb[:, gc, :], recipden[:, gc : gc + 1]
                    )
                nc.sync.dma_start(
                    out=probs_d[st["b"]].rearrange("(c p) n -> p c n", p=128),
                    in_=stage,
                )

            for b in range(BL):
                # ---- loads (gpsimd SWDGE casts f32 -> 16-bit) ----
                mask_t = pin.tile([128, 4, 512], bf16)
                nc.gpsimd.dma_start(
                    out=mask_t, in_=mask_d[b].rearrange("(c p) n -> p c n", p=128)
                )
                # x loads ride the SP/HWDGE queue (f32, no cast) so they run
                # in parallel with the mask's Pool/SWDGE casting load; the
                # PSUM->SBUF copy after the PE transpose does the fp16 cast.
                x_nodes = pin.tile([128, 4, 128], f32)
                nc.sync.dma_start(
                    out=x_nodes, in_=nodes_d[b].rearrange("(c p) e -> p c e", p=128)
                )
                x_q1 = pin.tile([128, 4, 128], f32)
                nc.sync.dma_start(
                    out=x_q1, in_=q1_d[b].rearrange("(c p) e -> p c e", p=128)
                )
                x_last = pin.tile([128, 4, 128], f32)
                nc.sync.dma_start(
                    out=x_last, in_=last_d[b].rearrange("(c p) e -> p c e", p=128)
                )

                # ---- transpose mask to [N, G] per n-chunk (bf16) ----
                maskT = pxt.tile([128, 4, 512], bf16)
                for c in range(4):
                    mp = ps_m.tile([128, 512], bf16, tag="m", name="mp")
                    for gc in range(4):
                        nc.tensor.transpose(
                            mp[:, gc * 128 : (gc + 1) * 128],
                            mask_t[:, gc, c * 128 : (c + 1) * 128],
                            ident_b,
                        )
                    nc.vector.tensor_copy(maskT[:, c, :], mp)

                # ---- transpose activations to [E, G] (fp16) ----
                def transpose_to(dst, src_nat):
                    tp = ps_m.tile([128, 512], f32, tag="m", name="tp")
                    for c in range(4):
                        nc.tensor.transpose(
                            tp[:, c * 128 : (c + 1) * 128], src_nat[:, c, :], ident_f
                        )
                    nc.vector.tensor_copy(dst, tp)

                nodesT = pxt.tile([128, 512], f16)
                transpose_to(nodesT, x_nodes)
                q1T = pxt.tile([128, 512], f16)
                transpose_to(q1T, x_q1)
                lastT = pxt.tile([128, 512], f16)
                transpose_to(lastT, x_last)

                # ---- projections (transposed/head-padded outputs, fp16) ----
                qT = []
                for g4 in range(2):  # head groups 0-3 / 4-7
                    ps = ps_m.tile([128, 512], f32, tag="m", name="psq")
                    nc.tensor.matmul(
                        ps, w_sb[f"Wq1p{g4}"], q1T, start=True, stop=False
                    )
                    nc.tensor.matmul(
                        ps, w_sb[f"Wqlp{g4}"], lastT, start=False, stop=True
                    )
                    t = pproj.tile([128, 512], f16, tag=f"qT{g4}", name="qt")
                    nc.vector.tensor_copy(t, ps)
                    qT.append(t)
                kT = []
                for g4 in range(2):
                    ps = ps_m.tile([128, 512], f32, tag="m", name="psk")
                    nc.tensor.matmul(
                        ps, w_sb[f"Wkp{g4}"], nodesT, start=True, stop=True
                    )
                    t = pproj.tile([128, 512], f16, tag=f"kT{g4}", name="kt")
                    nc.vector.tensor_copy(t, ps)
                    kT.append(t)
                # v natural [n, hd] scattered into v_aug 17-col head blocks
                v_aug = pproj.tile([128, 4, 136], f16, tag="vaug", name="v_aug")
                v_aug_blk = v_aug.rearrange("p c (h x) -> p c h x", x=17)
                nc.vector.memset(v_aug_blk[:, :, :, 16:17], 1.0)
                for c in range(4):
                    psv = ps_m.tile([128, 128], f32, tag="m", name="psv")
                    nc.tensor.matmul(
                        psv,
                        nodesT[:, c * 128 : (c + 1) * 128],
                        w_sb["Wv"],
                        start=True,
                        stop=True,
                    )
                    nc.vector.tensor_copy(
                        v_aug_blk[:, c, :, 0:16],
                        psv.rearrange("p (h d) -> p h d", d=16),
                    )

                # ---- attention: scoresT + mask -> exp -> flipped PV ----
                ua = ps_u.tile([128, 2, 136], f32, tag="ua", name="ua")
                ub = ps_u.tile([128, 2, 136], f32, tag="ub", name="ub")
                for half in range(2):  # heads 0-3 then 4-7
                    for hp in range(2):  # head pair within group
                        j0, j1 = 2 * hp, 2 * hp + 1  # pad-slot indices
                        expps = []
                        for c in range(4):  # n-chunks
                            sc = ps_score.tile(
                                [128, 1024], f32, tag="sc", name="sc"
                            )
                            for idx, j in enumerate((j0, j1)):
                                sl = sc[:, idx * 512 : (idx + 1) * 512]
                                nc.tensor.matmul(
                                    sl,
                                    ident_b,
                                    maskT[:, c, :],
                                    start=True,
                                    stop=False,
                                )
                                nc.tensor.matmul(
                                    sl,
                                    kT[half][
                                        32 * j : 32 * j + 16,
                                        c * 128 : (c + 1) * 128,
                                    ],
                                    qT[half][32 * j : 32 * j + 16, :],
                                    start=False,
                                    stop=True,
                                    tile_position=(32 * j, 0),
                                )
                            expp = pexp.tile([128, 1024], f16, tag="expp", name="ex")
                            nc.scalar.activation(expp, sc, AF.Exp)
                            expps.append(expp)
                        # flipped PV: stationary = exp^T g-block, moving = 17
                        # v_aug cols; output natural [g, 17h] accumulated in
                        # PSUM over the 4 n-chunks.
                        for idx, j in enumerate((j0, j1)):
                            vb = 17 * (half * 4 + j)
                            for gc in range(4):
                                ut = ua if gc < 2 else ub
                                gci = gc % 2
                                for c in range(4):
                                    nc.tensor.matmul(
                                        ut[:, gci, vb : vb + 17],
                                        expps[c][
                                            :,
                                            idx * 512
                                            + gc * 128 : idx * 512
                                            + (gc + 1) * 128,
                                        ],
                                        v_aug[:, c, vb : vb + 17],
                                        start=(c == 0),
                                        stop=(c == 3),
                                    )
                    # emit the previous batch's branch 2 mid-attention so the
                    # Act engine has ready work while this batch's PV ->
                    # normalize -> combine chain runs.
                    if half == 0 and prev:
                        branch2(prev)
                        prev = {}

                # ---- normalize per head, natural space ----
                # GPSIMD cannot read PSUM: stage u into SBUF first, then the
                # per-head scales can split across DVE and GPSIMD.
                u_sb = pmisc.tile([128, 4, 136], f32, tag="usb", name="u_sb")
                nc.vector.tensor_copy(u_sb[:, 0:2, :], ua)
                nc.vector.tensor_copy(u_sb[:, 2:4, :], ub)
                u_vw = u_sb.rearrange("p c (h x) -> p c h x", x=17)
                recip = pmisc.tile([128, 4, 8, 1], f32, tag="ra", name="recip")
                nc.vector.reciprocal(recip, u_vw[:, :, :, 16:17])
                u_norm = pmisc.tile([128, 4, 128], f16, tag="un", name="u_norm")
                # all scales on DVE: the Pool engine must stay free so the
                # next batch's SWDGE loads are never queued behind work that
                # waits on this batch's attention.
                for gc in range(4):
                    for h in range(8):
                        eng = nc.vector
                        eng.tensor_scalar_mul(
                            u_norm[:, gc, 16 * h : 16 * h + 16],
                            u_vw[:, gc, h, 0:16],
                            recip[:, gc, h, :],
                        )

                # ---- transpose u, combine + bias ----
                # NOTE: utp/mh_ps live in the "sc" ring (not "m") so that the
                # next batch's early transposes never wait on this batch's
                # late combine-phase consumers.
                utp = ps_score.tile([128, 512], f16, tag="sc", name="utp")
                for gc in range(4):
                    nc.tensor.transpose(
                        utp[:, gc * 128 : (gc + 1) * 128], u_norm[:, gc, :], ident
                    )
                uT = pmisc.tile([128, 512], f16, tag="uT", name="uT")
                nc.vector.tensor_copy(uT, utp)
                mh_ps = ps_score.tile([128, 512], f32, tag="sc", name="mh_ps")
                nc.tensor.matmul(mh_ps, w_sb["Wc"], uT, start=True, stop=True)
                mhT = pmisc.tile([128, 512], f16, tag="mhT", name="mhT")
                nc.vector.tensor_scalar_add(mhT, mh_ps, b_sb[:, 0:1])

                prev = {"b": b, "mhT": mhT, "nodesT": nodesT, "mask_t": mask_t}

            # drain the pipelined branch 2 for the last batch
            branch2(prev)

    if legalize:
        _legalize_waits(nc)
    return nc


def _prep_weights(inputs):
    def pad4(W):
        Wp0 = np.zeros((E, 128), np.float16)
        Wp1 = np.zeros((E, 128), np.float16)
        for j in range(4):
            Wp0[:, 32 * j : 32 * j + 16] = W[:, 16 * j : 16 * j + 16]
            Wp1[:, 32 * j : 32 * j + 16] = W[:, 64 + 16 * j : 64 + 16 * j + 16]
        return Wp0, Wp1

    s = np.float32(1.0 / np.sqrt(np.float32(D)))  # 0.25 folded into q weights
    Wq1p0, Wq1p1 = pad4(np.asarray(inputs["Wq_first"], np.float32) * s)
    Wqlp0, Wqlp1 = pad4(np.asarray(inputs["Wq_last"], np.float32) * s)
    Wkp0, Wkp1 = pad4(np.asarray(inputs["Wk"], np.float32))
    wd = {
        "Wq1p0": Wq1p0,
        "Wq1p1": Wq1p1,
        "Wqlp0": Wqlp0,
        "Wqlp1": Wqlp1,
        "Wkp0": Wkp0,
        "Wkp1": Wkp1,
        "Wv": np.asarray(inputs["Wv"], np.float16),
        "Wc": np.asarray(inputs["W_comb"], np.float16),
    }
    wcat = np.stack([wd[n] for n in WNAMES], axis=1)  # [E, 8, 128]
    return {
        "wcat": np.ascontiguousarray(wcat),
        "bc": np.asarray(inputs["b_comb"], np.float32).reshape(E, 1),
    }


def run(inputs, trace=False):
    from concourse.bass_utils import run_bass_kernel_spmd

    if "nc" not in _CACHE:
        _CACHE["nc"] = _build_nc()
    nc = _CACHE["nc"]

    w = _prep_weights(inputs)
    nodes = np.ascontiguousarray(np.asarray(inputs["encoded_nodes"], np.float32))
    q1 = np.ascontiguousarray(np.asarray(inputs["encoded_q1"], np.float32))
    last = np.ascontiguousarray(np.asarray(inputs["encoded_last_node"], np.float32))
    mask = np.ascontiguousarray(np.asarray(inputs["ninf_mask"], np.float32))

    in_maps = []
    for i in range(NCORES):
        sl = slice(i * BL, (i + 1) * BL)
        in_maps.append(
            {
                "nodes": nodes[sl],
                "q1": q1[sl],
                "last": last[sl],
                "mask": mask[sl],
                **w,
            }
        )
    try:
        res = run_bass_kernel_spmd(nc, in_maps, list(range(NCORES)), trace=trace)
    except Exception:
        # The first execution of a freshly compiled NEFF occasionally dies
        # with NRT_EXEC_UNIT_UNRECOVERABLE on this stack; a retry with the
        # cached NEFF has always succeeded.
        res = run_bass_kernel_spmd(nc, in_maps, list(range(NCORES)), trace=trace)
    out = np.concatenate(
        [res.results[i]["probs"].astype(np.float32) for i in range(NCORES)], axis=0
    )
    return out, res


def kernel(**inputs) -> np.ndarray:
    out, _ = run(inputs, trace=False)
    return out


# revision 17
# speedup vs baseline: 1.0325x; 1.0325x over previous
"""Trainium2 Bass kernel for nn_Decoder_14680198217759.

Multi-head attention decoder (B=32, G=N=512, E=128, H=8, D=16), pure data
parallel over 8 NeuronCores (4 batches/core).

v3 design notes (cost-model driven):
  - Act engine is the floor: exp of 8 heads x 512x512 per batch must run on
    the Activation engine (~16 x [128,1024] exps/batch, ~22us/batch).
    Everything else is arranged to hide under it.
  - All activations/weights flow as fp16 (mask as bf16, holds -1e9): PE
    streams at 1 cyc/col, DVE copies get the 2x 16-bit mode, DMA bytes halve.
  - Activations are loaded natural via gpsimd casting DMA (f32->fp16), then
    transposed E-major on the PE (4 transposes fused into one [128,512] PSUM
    tile, one copy out).
  - Scores are computed transposed per head pair into [128,1024] PSUM; the
    rank-3 mask rides as the leading matmul of each accumulation group
    (lhsT=I, rhs=maskT chunk). exp on Act -> fp16 SBUF.
  - PV is FLIPPED: lhsT = exp^T chunk (stationary, free weight load), rhs =
    v_aug 17-col block (16 v cols + ones col for the denominator): 17-col
    output streams instead of 512. Output lands NATURAL [g, 17h] in PSUM, so
    per-head normalization is per-partition tensor_scalar ops, no transpose
    dance. (GPSIMD cannot read PSUM: u is staged to SBUF first.)
  - Branch 2 mask is added into the s2 PSUM before tanh: tanh saturates to
    -1 for masked entries, exp(10*-1)=e-10 ~ 0 (error ~1e-5 relative).
    Denominators come free via activation accum_out.
  - Branch 2 of batch b-1 is software-pipelined into the middle of batch b's
    attention so the Act engine never waits for the PV->normalize->combine
    chain.
  - Output stored fp16, host casts to f32.
"""

import numpy as np

B, G, N, E, H, D = 32, 512, 512, 128, 8, 16
SQRT_E = 11.313708498984761
NCORES = 8
BL = B // NCORES  # batches per core

WNAMES = ["Wq1p0", "Wq1p1", "Wqlp0", "Wqlp1", "Wkp0", "Wkp1", "Wv", "Wc"]

_CACHE = {}


# --------------------------------------------------------------------------
# BIR wait legalization: this toolchain's walrus accepts at most ONE sem wait
# per instruction; Tile's scheduler can emit more (notably on the kernel-tail
# drain). Split excess waits onto same-engine NoOps placed directly before
# the offending instruction (same-queue program order keeps the semantics).
# --------------------------------------------------------------------------
def _legalize_waits(nc, max_waits=1):
    import concourse.mybir as mybir

    n_split = 0
    for f in nc.m.functions:
        for bb in f.blocks:
            out = []
            for ins in bb.instructions:
                si = ins.sync_info
                waits = list(si.on_wait) if si and si.on_wait else []
                if len(waits) > max_waits:
                    while len(waits) > max_waits:
                        chunk, waits = waits[:max_waits], waits[max_waits:]
                        nop = mybir.InstNoOp(
                            name=f"I-waitfix-{nc.next_id()}", ins=[], outs=[]
                        )
                        nop.engine = ins.engine
                        nop.sync_info = mybir.SyncInfo(on_wait=chunk, on_update=[])
                        out.append(nop)
                        n_split += 1
                    ins.sync_info = mybir.SyncInfo(
                        on_wait=waits, on_update=list(si.on_update or [])
                    )
                out.append(ins)
            bb.instructions[:] = out
    return n_split


def _build_nc(legalize=True):
    import concourse.bass as bass
    import concourse.mybir as mybir
    import concourse.tile as tile
    from concourse.masks import make_identity

    f32 = mybir.dt.float32
    f16 = mybir.dt.float16
    bf16 = mybir.dt.bfloat16
    AF = mybir.ActivationFunctionType

    nc = bass.Bass()

    nodes_d = nc.dram_tensor("nodes", [BL, N, E], f32, kind="ExternalInput")
    q1_d = nc.dram_tensor("q1", [BL, G, E], f32, kind="ExternalInput")
    last_d = nc.dram_tensor("last", [BL, G, E], f32, kind="ExternalInput")
    mask_d = nc.dram_tensor("mask", [BL, G, N], f32, kind="ExternalInput")
    wcat_d = nc.dram_tensor("wcat", [E, 8, 128], f16, kind="ExternalInput")
    b_d = nc.dram_tensor("bc", [E, 1], f32, kind="ExternalInput")
    probs_d = nc.dram_tensor("probs", [BL, G, N], f16, kind="ExternalOutput")

    with tile.TileContext(nc) as tc:
        import contextlib

        with contextlib.ExitStack() as ctx:
            pw = ctx.enter_context(tc.tile_pool(name="pw", bufs=1))
            pin = ctx.enter_context(tc.tile_pool(name="pin", bufs=2))
            pxt = ctx.enter_context(tc.tile_pool(name="pxt", bufs=2))
            pproj = ctx.enter_context(tc.tile_pool(name="pproj", bufs=2))
            pexp = ctx.enter_context(tc.tile_pool(name="pexp", bufs=6))
            pmisc = ctx.enter_context(tc.tile_pool(name="pmisc", bufs=2))
            pstage = ctx.enter_context(tc.tile_pool(name="pstage", bufs=2))
            ps_score = ctx.enter_context(
                tc.tile_pool(name="ps_score", bufs=2, space="PSUM")
            )
            ps_u = ctx.enter_context(tc.tile_pool(name="ps_u", bufs=1, space="PSUM"))
            ps_m = ctx.enter_context(tc.tile_pool(name="ps_m", bufs=2, space="PSUM"))

            # ---- constants / weights (one batched DMA) ----
            ident = pw.tile([128, 128], f16)
            make_identity(nc, ident)
            ident_b = pw.tile([128, 128], bf16)
            make_identity(nc, ident_b)
            ident_f = pw.tile([128, 128], f32)
            make_identity(nc, ident_f)
            wcat = pw.tile([128, 8, 128], f16)
            nc.sync.dma_start(out=wcat, in_=wcat_d[:, :, :])
            w_sb = {n: wcat[:, i, :] for i, n in enumerate(WNAMES)}
            b_sb = pw.tile([128, 1], f32)
            nc.sync.dma_start(out=b_sb, in_=b_d[:, :])

            # state carried between loop iterations for the software-pipelined
            # branch 2 (runs one batch behind the attention stream)
            prev = {}

            def branch2(st):
                t_sb = pstage.tile([128, 4, 512], f16, tag="tsb", name="t_sb")
                for gp in range(2):
                    s2 = ps_score.tile([128, 1024], f32, tag="sc", name="s2")
                    for k2 in range(2):
                        gc = 2 * gp + k2
                        sl = s2[:, k2 * 512 : (k2 + 1) * 512]
                        nc.tensor.matmul(
                            sl,
                            st["mhT"][:, gc * 128 : (gc + 1) * 128],
                            st["nodesT"],
                            start=True,
                            stop=False,
                        )
                        nc.tensor.matmul(
                            sl,
                            ident_b,
                            st["mask_t"][:, gc, :],
                            start=False,
                            stop=True,
                        )
                    nc.scalar.activation(
                        t_sb[:, 2 * gp : 2 * gp + 2, :],
                        s2,
                        AF.Tanh,
                        scale=1.0 / SQRT_E,
                    )
                den = pmisc.tile([128, 4], f32, tag="den", name="den")
                p_sb = pstage.tile([128, 4, 512], f16, tag="psb", name="p_sb")
                for gc in range(4):
                    nc.scalar.activation(
                        p_sb[:, gc, :],
                        t_sb[:, gc, :],
                        AF.Exp,
                        scale=10.0,
                        accum_out=den[:, gc : gc + 1],
                    )
                recipden = pmisc.tile([128, 4], f32, tag="rd", name="recipden")
                nc.vector.reciprocal(recipden, den)
                stage = pstage.tile([128, 4, 512], f16, tag="stg", name="stage")
                for gc in range(4):
                    eng = nc.vector if (gc % 2 == 0) else nc.gpsimd
                    eng.tensor_scalar_mul(
                        stage[:, gc, :], p_sb[:, gc, :], recipden[:, gc : gc + 1]
                    )
                nc.sync.dma_start(
                    out=probs_d[st["b"]].rearrange("(c p) n -> p c n", p=128),
                    in_=stage,
                )

            def emit_loads(b):
                # mask on the Pool/SWDGE casting queue; x on SP/HWDGE (f32,
                # no cast) so the two queues run in parallel. Loads for batch
                # b+1 are emitted BEFORE norm(b) so the Pool queue never
                # parks them behind scales that wait on batch b's attention.
                mask_t = pin.tile([128, 4, 512], bf16, tag="mk", name="mask_t")
                nc.gpsimd.dma_start(
                    out=mask_t, in_=mask_d[b].rearrange("(c p) n -> p c n", p=128)
                )
                x_nodes = pin.tile([128, 4, 128], f32, tag="xn", name="x_nodes")
                nc.sync.dma_start(
                    out=x_nodes, in_=nodes_d[b].rearrange("(c p) e -> p c e", p=128)
                )
                x_q1 = pin.tile([128, 4, 128], f32, tag="xq", name="x_q1")
                nc.sync.dma_start(
                    out=x_q1, in_=q1_d[b].rearrange("(c p) e -> p c e", p=128)
                )
                x_last = pin.tile([128, 4, 128], f32, tag="xl", name="x_last")
                nc.sync.dma_start(
                    out=x_last, in_=last_d[b].rearrange("(c p) e -> p c e", p=128)
                )
                return mask_t, x_nodes, x_q1, x_last

            loaded = emit_loads(0)
            for b in range(BL):
                mask_t, x_nodes, x_q1, x_last = loaded

                # ---- transpose mask to [N, G] per n-chunk (bf16) ----
                maskT = pxt.tile([128, 4, 512], bf16)
                for c in range(4):
                    mp = ps_m.tile([128, 512], bf16, tag="m", name="mp")
                    for gc in range(4):
                        nc.tensor.transpose(
                            mp[:, gc * 128 : (gc + 1) * 128],
                            mask_t[:, gc, c * 128 : (c + 1) * 128],
                            ident_b,
                        )
                    nc.vector.tensor_copy(maskT[:, c, :], mp)

                # ---- transpose activations to [E, G] (fp16) ----
                def transpose_to(dst, src_nat):
                    tp = ps_m.tile([128, 512], f32, tag="m", name="tp")
                    for c in range(4):
                        nc.tensor.transpose(
                            tp[:, c * 128 : (c + 1) * 128], src_nat[:, c, :], ident_f
                        )
                    nc.vector.tensor_copy(dst, tp)

                nodesT = pxt.tile([128, 512], f16)
                transpose_to(nodesT, x_nodes)
                q1T = pxt.tile([128, 512], f16)
                transpose_to(q1T, x_q1)
                lastT = pxt.tile([128, 512], f16)
                transpose_to(lastT, x_last)

                # ---- projections (transposed/head-padded outputs, fp16) ----
                qT = []
                for g4 in range(2):  # head groups 0-3 / 4-7
                    ps = ps_m.tile([128, 512], f32, tag="m", name="psq")
                    nc.tensor.matmul(
                        ps, w_sb[f"Wq1p{g4}"], q1T, start=True, stop=False
                    )
                    nc.tensor.matmul(
                        ps, w_sb[f"Wqlp{g4}"], lastT, start=False, stop=True
                    )
                    t = pproj.tile([128, 512], f16, tag=f"qT{g4}", name="qt")
                    nc.vector.tensor_copy(t, ps)
                    qT.append(t)
                kT = []
                for g4 in range(2):
                    ps = ps_m.tile([128, 512], f32, tag="m", name="psk")
                    nc.tensor.matmul(
                        ps, w_sb[f"Wkp{g4}"], nodesT, start=True, stop=True
                    )
                    t = pproj.tile([128, 512], f16, tag=f"kT{g4}", name="kt")
                    nc.vector.tensor_copy(t, ps)
                    kT.append(t)
                # v natural [n, hd] scattered into v_aug 17-col head blocks
                v_aug = pproj.tile([128, 4, 136], f16, tag="vaug", name="v_aug")
                v_aug_blk = v_aug.rearrange("p c (h x) -> p c h x", x=17)
                nc.vector.memset(v_aug_blk[:, :, :, 16:17], 1.0)
                for c in range(4):
                    psv = ps_m.tile([128, 128], f32, tag="m", name="psv")
                    nc.tensor.matmul(
                        psv,
                        nodesT[:, c * 128 : (c + 1) * 128],
                        w_sb["Wv"],
                        start=True,
                        stop=True,
                    )
                    nc.vector.tensor_copy(
                        v_aug_blk[:, c, :, 0:16],
                        psv.rearrange("p (h d) -> p h d", d=16),
                    )

                # ---- attention: scoresT + mask -> exp -> flipped PV ----
                ua = ps_u.tile([128, 2, 136], f32, tag="ua", name="ua")
                ub = ps_u.tile([128, 2, 136], f32, tag="ub", name="ub")
                for half in range(2):  # heads 0-3 then 4-7
                    for hp in range(2):  # head pair within group
                        j0, j1 = 2 * hp, 2 * hp + 1  # pad-slot indices
                        expps = []
                        for c in range(4):  # n-chunks
                            sc = ps_score.tile(
                                [128, 1024], f32, tag="sc", name="sc"
                            )
                            for idx, j in enumerate((j0, j1)):
                                sl = sc[:, idx * 512 : (idx + 1) * 512]
                                nc.tensor.matmul(
                                    sl,
                                    ident_b,
                                    maskT[:, c, :],
                                    start=True,
                                    stop=False,
                                )
                                nc.tensor.matmul(
                                    sl,
                                    kT[half][
                                        32 * j : 32 * j + 16,
                                        c * 128 : (c + 1) * 128,
                                    ],
                                    qT[half][32 * j : 32 * j + 16, :],
                                    start=False,
                                    stop=True,
                                    tile_position=(32 * j, 0),
                                )
                            expp = pexp.tile([128, 1024], f16, tag="expp", name="ex")
                            nc.scalar.activation(expp, sc, AF.Exp)
                            expps.append(expp)
                        # flipped PV: stationary = exp^T g-block, moving = 17
                        # v_aug cols; output natural [g, 17h] accumulated in
                        # PSUM over the 4 n-chunks.
                        for idx, j in enumerate((j0, j1)):
                            vb = 17 * (half * 4 + j)
                            for gc in range(4):
                                ut = ua if gc < 2 else ub
                                gci = gc % 2
                                for c in range(4):
                                    nc.tensor.matmul(
                                        ut[:, gci, vb : vb + 17],
                                        expps[c][
                                            :,
                                            idx * 512
                                            + gc * 128 : idx * 512
                                            + (gc + 1) * 128,
                                        ],
                                        v_aug[:, c, vb : vb + 17],
                                        start=(c == 0),
                                        stop=(c == 3),
                                    )
                    # emit the previous batch's branch 2 mid-attention so the
                    # Act engine has ready work while this batch's PV ->
                    # normalize -> combine chain runs.
                    if half == 0 and prev:
                        branch2(prev)
                        prev = {}

                # prefetch next batch's inputs before any norm-phase work
                # lands in the Pool queue
                if b + 1 < BL:
                    loaded = emit_loads(b + 1)

                # ---- normalize per head, natural space ----
                # GPSIMD cannot read PSUM: stage u into SBUF first, then the
                # per-head scales can split across DVE and GPSIMD.
                u_sb = pmisc.tile([128, 4, 136], f32, tag="usb", name="u_sb")
                nc.vector.tensor_copy(u_sb[:, 0:2, :], ua)
                nc.vector.tensor_copy(u_sb[:, 2:4, :], ub)
                u_vw = u_sb.rearrange("p c (h x) -> p c h x", x=17)
                recip = pmisc.tile([128, 4, 8, 1], f32, tag="ra", name="recip")
                nc.vector.reciprocal(recip, u_vw[:, :, :, 16:17])
                u_norm = pmisc.tile([128, 4, 128], f16, tag="un", name="u_norm")
                for gc in range(4):
                    for h in range(8):
                        eng = nc.vector if (h % 2 == 0) else nc.gpsimd
                        eng.tensor_scalar_mul(
                            u_norm[:, gc, 16 * h : 16 * h + 16],
                            u_vw[:, gc, h, 0:16],
                            recip[:, gc, h, :],
                        )

                # ---- transpose u, combine + bias ----
                # NOTE: utp/mh_ps live in the "sc" ring (not "m") so that the
                # next batch's early transposes never wait on this batch's
                # late combine-phase consumers.
                utp = ps_score.tile([128, 512], f16, tag="sc", name="utp")
                for gc in range(4):
                    nc.tensor.transpose(
                        utp[:, gc * 128 : (gc + 1) * 128], u_norm[:, gc, :], ident
                    )
                uT = pmisc.tile([128, 512], f16, tag="uT", name="uT")
                nc.vector.tensor_copy(uT, utp)
                mh_ps = ps_score.tile([128, 512], f32, tag="sc", name="mh_ps")
                nc.tensor.matmul(mh_ps, w_sb["Wc"], uT, start=True, stop=True)
                mhT = pmisc.tile([128, 512], f16, tag="mhT", name="mhT")
                nc.vector.tensor_scalar_add(mhT, mh_ps, b_sb[:, 0:1])

                prev = {"b": b, "mhT": mhT, "nodesT": nodesT, "mask_t": mask_t}

            # drain the pipelined branch 2 for the last batch
            branch2(prev)

    if legalize:
        _legalize_waits(nc)
    return nc


def _prep_weights(inputs):
    def pad4(W):
        Wp0 = np.zeros((E, 128), np.float16)
        Wp1 = np.zeros((E, 128), np.float16)
        for j in range(4):
            Wp0[:, 32 * j : 32 * j + 16] = W[:, 16 * j : 16 * j + 16]
            Wp1[:, 32 * j : 32 * j + 16] = W[:, 64 + 16 * j : 64 + 16 * j + 16]
        return Wp0, Wp1

    s = np.float32(1.0 / np.sqrt(np.float32(D)))  # 0.25 folded into q weights
    Wq1p0, Wq1p1 = pad4(np.asarray(inputs["Wq_first"], np.float32) * s)
    Wqlp0, Wqlp1 = pad4(np.asarray(inputs["Wq_last"], np.float32) * s)
    Wkp0, Wkp1 = pad4(np.asarray(inputs["Wk"], np.float32))
    wd = {
        "Wq1p0": Wq1p0,
        "Wq1p1": Wq1p1,
        "Wqlp0": Wqlp0,
        "Wqlp1": Wqlp1,
        "Wkp0": Wkp0,
        "Wkp1": Wkp1,
        "Wv": np.asarray(inputs["Wv"], np.float16),
        "Wc": np.asarray(inputs["W_comb"], np.float16),
    }
    wcat = np.stack([wd[n] for n in WNAMES], axis=1)  # [E, 8, 128]
    return {
        "wcat": np.ascontiguousarray(wcat),
        "bc": np.asarray(inputs["b_comb"], np.float32).reshape(E, 1),
    }


def run(inputs, trace=False):
    from concourse.bass_utils import run_bass_kernel_spmd

    if "nc" not in _CACHE:
        _CACHE["nc"] = _build_nc()
    nc = _CACHE["nc"]

    w = _prep_weights(inputs)
    nodes = np.ascontiguousarray(np.asarray(inputs["encoded_nodes"], np.float32))
    q1 = np.ascontiguousarray(np.asarray(inputs["encoded_q1"], np.float32))
    last = np.ascontiguousarray(np.asarray(inputs["encoded_last_node"], np.float32))
    mask = np.ascontiguousarray(np.asarray(inputs["ninf_mask"], np.float32))

    in_maps = []
    for i in range(NCORES):
        sl = slice(i * BL, (i + 1) * BL)
        in_maps.append(
            {
                "nodes": nodes[sl],
                "q1": q1[sl],
                "last": last[sl],
                "mask": mask[sl],
                **w,
            }
        )
    try:
        res = run_bass_kernel_spmd(nc, in_maps, list(range(NCORES)), trace=trace)
    except Exception:
        # The first execution of a freshly compiled NEFF occasionally dies
        # with NRT_EXEC_UNIT_UNRECOVERABLE on this stack; a retry with the
        # cached NEFF has always succeeded.
        res = run_bass_kernel_spmd(nc, in_maps, list(range(NCORES)), trace=trace)
    out = np.concatenate(
        [res.results[i]["probs"].astype(np.float32) for i in range(NCORES)], axis=0
    )
    return out, res


def kernel(**inputs) -> np.ndarray:
    out, _ = run(inputs, trace=False)
    return out


# revision 19
# speedup vs baseline: 1.0523x; 1.0191x over previous
"""Trainium2 Bass kernel for nn_Decoder_14680198217759.

Multi-head attention decoder (B=32, G=N=512, E=128, H=8, D=16), pure data
parallel over 8 NeuronCores (4 batches/core).

v3 design notes (cost-model driven):
  - Act engine is the floor: exp of 8 heads x 512x512 per batch must run on
    the Activation engine (~16 x [128,1024] exps/batch, ~22us/batch).
    Everything else is arranged to hide under it.
  - All activations/weights flow as fp16 (mask as bf16, holds -1e9): PE
    streams at 1 cyc/col, DVE copies get the 2x 16-bit mode, DMA bytes halve.
  - Activations are loaded natural via gpsimd casting DMA (f32->fp16), then
    transposed E-major on the PE (4 transposes fused into one [128,512] PSUM
    tile, one copy out).
  - Scores are computed transposed per head pair into [128,1024] PSUM; the
    rank-3 mask rides as the leading matmul of each accumulation group
    (lhsT=I, rhs=maskT chunk). exp on Act -> fp16 SBUF.
  - PV is FLIPPED: lhsT = exp^T chunk (stationary, free weight load), rhs =
    v_aug 17-col block (16 v cols + ones col for the denominator): 17-col
    output streams instead of 512. Output lands NATURAL [g, 17h] in PSUM, so
    per-head normalization is per-partition tensor_scalar ops, no transpose
    dance. (GPSIMD cannot read PSUM: u is staged to SBUF first.)
  - Branch 2 mask is added into the s2 PSUM before tanh: tanh saturates to
    -1 for masked entries, exp(10*-1)=e-10 ~ 0 (error ~1e-5 relative).
    Denominators come free via activation accum_out.
  - Branch 2 of batch b-1 is software-pipelined into the middle of batch b's
    attention so the Act engine never waits for the PV->normalize->combine
    chain.
  - Output stored fp16, host casts to f32.
"""

import numpy as np

B, G, N, E, H, D = 32, 512, 512, 128, 8, 16
SQRT_E = 11.313708498984761
NCORES = 8
BL = B // NCORES  # batches per core

WNAMES = ["Wq1p0", "Wq1p1", "Wqlp0", "Wqlp1", "Wkp0", "Wkp1", "Wv", "Wc"]

_CACHE = {}


# --------------------------------------------------------------------------
# BIR wait legalization: this toolchain's walrus accepts at most ONE sem wait
# per instruction; Tile's scheduler can emit more (notably on the kernel-tail
# drain). Split excess waits onto same-engine NoOps placed directly before
# the offending instruction (same-queue program order keeps the semantics).
# --------------------------------------------------------------------------
def _legalize_waits(nc, max_waits=1):
    import concourse.mybir as mybir

    n_split = 0
    for f in nc.m.functions:
        for bb in f.blocks:
            out = []
            for ins in bb.instructions:
                si = ins.sync_info
                waits = list(si.on_wait) if si and si.on_wait else []
                if len(waits) > max_waits:
                    while len(waits) > max_waits:
                        chunk, waits = waits[:max_waits], waits[max_waits:]
                        nop = mybir.InstNoOp(
                            name=f"I-waitfix-{nc.next_id()}", ins=[], outs=[]
                        )
                        nop.engine = ins.engine
                        nop.sync_info = mybir.SyncInfo(on_wait=chunk, on_update=[])
                        out.append(nop)
                        n_split += 1
                    ins.sync_info = mybir.SyncInfo(
                        on_wait=waits, on_update=list(si.on_update or [])
                    )
                out.append(ins)
            bb.instructions[:] = out
    return n_split


def _build_nc(legalize=True):
    import concourse.bass as bass
    import concourse.mybir as mybir
    import concourse.tile as tile
    from concourse.masks import make_identity

    f32 = mybir.dt.float32
    f16 = mybir.dt.float16
    bf16 = mybir.dt.bfloat16
    AF = mybir.ActivationFunctionType

    nc = bass.Bass()

    nodes_d = nc.dram_tensor("nodes", [BL, N, E], f32, kind="ExternalInput")
    q1_d = nc.dram_tensor("q1", [BL, G, E], f32, kind="ExternalInput")
    last_d = nc.dram_tensor("last", [BL, G, E], f32, kind="ExternalInput")
    mask_d = nc.dram_tensor("mask", [BL, G, N], f32, kind="ExternalInput")
    wcat_d = nc.dram_tensor("wcat", [E, 8, 128], f16, kind="ExternalInput")
    b_d = nc.dram_tensor("bc", [E, 1], f32, kind="ExternalInput")
    probs_d = nc.dram_tensor("probs", [BL, G, N], f16, kind="ExternalOutput")

    with tile.TileContext(nc) as tc:
        import contextlib

        with contextlib.ExitStack() as ctx:
            pw = ctx.enter_context(tc.tile_pool(name="pw", bufs=1))
            pin = ctx.enter_context(tc.tile_pool(name="pin", bufs=2))
            pxt = ctx.enter_context(tc.tile_pool(name="pxt", bufs=2))
            pproj = ctx.enter_context(tc.tile_pool(name="pproj", bufs=2))
            pexp = ctx.enter_context(tc.tile_pool(name="pexp", bufs=6))
            pmisc = ctx.enter_context(tc.tile_pool(name="pmisc", bufs=2))
            pstage = ctx.enter_context(tc.tile_pool(name="pstage", bufs=2))
            ps_score = ctx.enter_context(
                tc.tile_pool(name="ps_score", bufs=2, space="PSUM")
            )
            ps_u = ctx.enter_context(tc.tile_pool(name="ps_u", bufs=1, space="PSUM"))
            ps_m = ctx.enter_context(tc.tile_pool(name="ps_m", bufs=2, space="PSUM"))

            # ---- constants / weights (one batched DMA) ----
            ident = pw.tile([128, 128], f16)
            make_identity(nc, ident)
            ident_b = pw.tile([128, 128], bf16)
            make_identity(nc, ident_b)
            ident_f = pw.tile([128, 128], f32)
            make_identity(nc, ident_f)
            wcat = pw.tile([128, 8, 128], f16)
            nc.sync.dma_start(out=wcat, in_=wcat_d[:, :, :])
            w_sb = {n: wcat[:, i, :] for i, n in enumerate(WNAMES)}
            b_sb = pw.tile([128, 1], f32)
            nc.sync.dma_start(out=b_sb, in_=b_d[:, :])

            # state carried between loop iterations for the software-pipelined
            # branch 2 (runs one batch behind the attention stream)
            prev = {}

            def branch2(st):
                t_sb = pstage.tile([128, 4, 512], f16, tag="tsb", name="t_sb")
                for gp in range(2):
                    s2 = ps_score.tile([128, 1024], f32, tag="sc", name="s2")
                    for k2 in range(2):
                        gc = 2 * gp + k2
                        sl = s2[:, k2 * 512 : (k2 + 1) * 512]
                        nc.tensor.matmul(
                            sl,
                            st["mhT"][:, gc * 128 : (gc + 1) * 128],
                            st["nodesT"],
                            start=True,
                            stop=False,
                        )
                        nc.tensor.matmul(
                            sl,
                            ident_b,
                            st["mask_t"][:, gc, :],
                            start=False,
                            stop=True,
                        )
                    nc.scalar.activation(
                        t_sb[:, 2 * gp : 2 * gp + 2, :],
                        s2,
                        AF.Tanh,
                        scale=1.0 / SQRT_E,
                    )
                den = pmisc.tile([128, 4], f32, tag="den", name="den")
                p_sb = pstage.tile([128, 4, 512], f16, tag="psb", name="p_sb")
                for gc in range(4):
                    nc.scalar.activation(
                        p_sb[:, gc, :],
                        t_sb[:, gc, :],
                        AF.Exp,
                        scale=10.0,
                        accum_out=den[:, gc : gc + 1],
                    )
                recipden = pmisc.tile([128, 4], f32, tag="rd", name="recipden")
                nc.vector.reciprocal(recipden, den)
                stage = pstage.tile([128, 4, 512], f16, tag="stg", name="stage")
                for gc in range(4):
                    eng = nc.vector if (gc % 2 == 0) else nc.gpsimd
                    eng.tensor_scalar_mul(
                        stage[:, gc, :], p_sb[:, gc, :], recipden[:, gc : gc + 1]
                    )
                nc.sync.dma_start(
                    out=probs_d[st["b"]].rearrange("(c p) n -> p c n", p=128),
                    in_=stage,
                )

            def emit_loads(b):
                # mask on the Pool/SWDGE casting queue; x on SP/HWDGE (f32,
                # no cast) so the two queues run in parallel. Loads for batch
                # b+1 are emitted BEFORE norm(b) so the Pool queue never
                # parks them behind scales that wait on batch b's attention.
                mask_t = pin.tile([128, 4, 512], bf16, tag="mk", name="mask_t")
                nc.gpsimd.dma_start(
                    out=mask_t, in_=mask_d[b].rearrange("(c p) n -> p c n", p=128)
                )
                x_nodes = pin.tile([128, 4, 128], f16, tag="xn", name="x_nodes")
                nc.gpsimd.dma_start(
                    out=x_nodes, in_=nodes_d[b].rearrange("(c p) e -> p c e", p=128)
                )
                x_q1 = pin.tile([128, 4, 128], f16, tag="xq", name="x_q1")
                nc.gpsimd.dma_start(
                    out=x_q1, in_=q1_d[b].rearrange("(c p) e -> p c e", p=128)
                )
                x_last = pin.tile([128, 4, 128], f16, tag="xl", name="x_last")
                nc.gpsimd.dma_start(
                    out=x_last, in_=last_d[b].rearrange("(c p) e -> p c e", p=128)
                )
                return mask_t, x_nodes, x_q1, x_last

            loaded = emit_loads(0)
            for b in range(BL):
                mask_t, x_nodes, x_q1, x_last = loaded

                # ---- transpose mask to [N, G] per n-chunk (bf16) ----
                maskT = pxt.tile([128, 4, 512], bf16)
                for c in range(4):
                    mp = ps_m.tile([128, 512], bf16, tag="m", name="mp")
                    for gc in range(4):
                        nc.tensor.transpose(
                            mp[:, gc * 128 : (gc + 1) * 128],
                            mask_t[:, gc, c * 128 : (c + 1) * 128],
                            ident_b,
                        )
                    nc.vector.tensor_copy(maskT[:, c, :], mp)

                # ---- transpose activations to [E, G] (fp16) ----
                def transpose_to(dst, src_nat):
                    tp = ps_m.tile([128, 512], f16, tag="m", name="tp")
                    for c in range(4):
                        nc.tensor.transpose(
                            tp[:, c * 128 : (c + 1) * 128], src_nat[:, c, :], ident
                        )
                    nc.vector.tensor_copy(dst, tp)

                nodesT = pxt.tile([128, 512], f16)
                transpose_to(nodesT, x_nodes)
                q1T = pxt.tile([128, 512], f16)
                transpose_to(q1T, x_q1)
                lastT = pxt.tile([128, 512], f16)
                transpose_to(lastT, x_last)

                # ---- projections (transposed/head-padded outputs, fp16) ----
                qT = []
                for g4 in range(2):  # head groups 0-3 / 4-7
                    ps = ps_m.tile([128, 512], f32, tag="m", name="psq")
                    nc.tensor.matmul(
                        ps, w_sb[f"Wq1p{g4}"], q1T, start=True, stop=False
                    )
                    nc.tensor.matmul(
                        ps, w_sb[f"Wqlp{g4}"], lastT, start=False, stop=True
                    )
                    t = pproj.tile([128, 512], f16, tag=f"qT{g4}", name="qt")
                    nc.vector.tensor_copy(t, ps)
                    qT.append(t)
                kT = []
                for g4 in range(2):
                    ps = ps_m.tile([128, 512], f32, tag="m", name="psk")
                    nc.tensor.matmul(
                        ps, w_sb[f"Wkp{g4}"], nodesT, start=True, stop=True
                    )
                    t = pproj.tile([128, 512], f16, tag=f"kT{g4}", name="kt")
                    nc.vector.tensor_copy(t, ps)
                    kT.append(t)
                # v natural [n, hd] scattered into v_aug 17-col head blocks
                v_aug = pproj.tile([128, 4, 136], f16, tag="vaug", name="v_aug")
                v_aug_blk = v_aug.rearrange("p c (h x) -> p c h x", x=17)
                nc.vector.memset(v_aug_blk[:, :, :, 16:17], 1.0)
                for c in range(4):
                    psv = ps_m.tile([128, 128], f32, tag="m", name="psv")
                    nc.tensor.matmul(
                        psv,
                        nodesT[:, c * 128 : (c + 1) * 128],
                        w_sb["Wv"],
                        start=True,
                        stop=True,
                    )
                    nc.vector.tensor_copy(
                        v_aug_blk[:, c, :, 0:16],
                        psv.rearrange("p (h d) -> p h d", d=16),
                    )

                # ---- attention: scoresT + mask -> exp -> flipped PV ----
                ua = ps_u.tile([128, 2, 136], f32, tag="ua", name="ua")
                ub = ps_u.tile([128, 2, 136], f32, tag="ub", name="ub")
                for half in range(2):  # heads 0-3 then 4-7
                    for hp in range(2):  # head pair within group
                        j0, j1 = 2 * hp, 2 * hp + 1  # pad-slot indices
                        expps = []
                        for c in range(4):  # n-chunks
                            sc = ps_score.tile(
                                [128, 1024], f32, tag="sc", name="sc"
                            )
                            for idx, j in enumerate((j0, j1)):
                                sl = sc[:, idx * 512 : (idx + 1) * 512]
                                nc.tensor.matmul(
                                    sl,
                                    ident_b,
                                    maskT[:, c, :],
                                    start=True,
                                    stop=False,
                                )
                                nc.tensor.matmul(
                                    sl,
                                    kT[half][
                                        32 * j : 32 * j + 16,
                                        c * 128 : (c + 1) * 128,
                                    ],
                                    qT[half][32 * j : 32 * j + 16, :],
                                    start=False,
                                    stop=True,
                                    tile_position=(32 * j, 0),
                                )
                            expp = pexp.tile([128, 1024], f16, tag="expp", name="ex")
                            nc.scalar.activation(expp, sc, AF.Exp)
                            expps.append(expp)
                        # flipped PV: stationary = exp^T g-block, moving = 17
                        # v_aug cols; output natural [g, 17h] accumulated in
                        # PSUM over the 4 n-chunks.
                        for idx, j in enumerate((j0, j1)):
                            vb = 17 * (half * 4 + j)
                            for gc in range(4):
                                ut = ua if gc < 2 else ub
                                gci = gc % 2
                                for c in range(4):
                                    nc.tensor.matmul(
                                        ut[:, gci, vb : vb + 17],
                                        expps[c][
                                            :,
                                            idx * 512
                                            + gc * 128 : idx * 512
                                            + (gc + 1) * 128,
                                        ],
                                        v_aug[:, c, vb : vb + 17],
                                        start=(c == 0),
                                        stop=(c == 3),
                                    )
                    # emit the previous batch's branch 2 mid-attention so the
                    # Act engine has ready work while this batch's PV ->
                    # normalize -> combine chain runs.
                    if half == 0 and prev:
                        branch2(prev)
                        prev = {}

                # prefetch next batch's inputs before any norm-phase work
                # lands in the Pool queue
                if b + 1 < BL:
                    loaded = emit_loads(b + 1)

                # ---- normalize per head, natural space ----
                # GPSIMD cannot read PSUM: stage u into SBUF first, then the
                # per-head scales can split across DVE and GPSIMD.
                u_sb = pmisc.tile([128, 4, 136], f32, tag="usb", name="u_sb")
                nc.vector.tensor_copy(u_sb[:, 0:2, :], ua)
                nc.vector.tensor_copy(u_sb[:, 2:4, :], ub)
                u_vw = u_sb.rearrange("p c (h x) -> p c h x", x=17)
                recip = pmisc.tile([128, 4, 8, 1], f32, tag="ra", name="recip")
                nc.vector.reciprocal(recip, u_vw[:, :, :, 16:17])
                u_norm = pmisc.tile([128, 4, 128], f16, tag="un", name="u_norm")
                for gc in range(4):
                    for h in range(8):
                        eng = nc.vector if (h % 2 == 0) else nc.gpsimd
                        eng.tensor_scalar_mul(
                            u_norm[:, gc, 16 * h : 16 * h + 16],
                            u_vw[:, gc, h, 0:16],
                            recip[:, gc, h, :],
                        )

                # ---- transpose u, combine + bias ----
                # NOTE: utp/mh_ps live in the "sc" ring (not "m") so that the
                # next batch's early transposes never wait on this batch's
                # late combine-phase consumers.
                utp = ps_score.tile([128, 512], f16, tag="sc", name="utp")
                for gc in range(4):
                    nc.tensor.transpose(
                        utp[:, gc * 128 : (gc + 1) * 128], u_norm[:, gc, :], ident
                    )
                uT = pmisc.tile([128, 512], f16, tag="uT", name="uT")
                nc.vector.tensor_copy(uT, utp)
                mh_ps = ps_score.tile([128, 512], f32, tag="sc", name="mh_ps")
                nc.tensor.matmul(mh_ps, w_sb["Wc"], uT, start=True, stop=True)
                mhT = pmisc.tile([128, 512], f16, tag="mhT", name="mhT")
                nc.vector.tensor_scalar_add(mhT, mh_ps, b_sb[:, 0:1])

                prev = {"b": b, "mhT": mhT, "nodesT": nodesT, "mask_t": mask_t}

            # drain the pipelined branch 2 for the last batch
            branch2(prev)

    if legalize:
        _legalize_waits(nc)
    return nc


def _prep_weights(inputs):
    def pad4(W):
        Wp0 = np.zeros((E, 128), np.float16)
        Wp1 = np.zeros((E, 128), np.float16)
        for j in range(4):
            Wp0[:, 32 * j : 32 * j + 16] = W[:, 16 * j : 16 * j + 16]
            Wp1[:, 32 * j : 32 * j + 16] = W[:, 64 + 16 * j : 64 + 16 * j + 16]
        return Wp0, Wp1

    s = np.float32(1.0 / np.sqrt(np.float32(D)))  # 0.25 folded into q weights
    Wq1p0, Wq1p1 = pad4(np.asarray(inputs["Wq_first"], np.float32) * s)
    Wqlp0, Wqlp1 = pad4(np.asarray(inputs["Wq_last"], np.float32) * s)
    Wkp0, Wkp1 = pad4(np.asarray(inputs["Wk"], np.float32))
    wd = {
        "Wq1p0": Wq1p0,
        "Wq1p1": Wq1p1,
        "Wqlp0": Wqlp0,
        "Wqlp1": Wqlp1,
        "Wkp0": Wkp0,
        "Wkp1": Wkp1,
        "Wv": np.asarray(inputs["Wv"], np.float16),
        "Wc": np.asarray(inputs["W_comb"], np.float16),
    }
    wcat = np.stack([wd[n] for n in WNAMES], axis=1)  # [E, 8, 128]
    return {
        "wcat": np.ascontiguousarray(wcat),
        "bc": np.asarray(inputs["b_comb"], np.float32).reshape(E, 1),
    }


def run(inputs, trace=False):
    from concourse.bass_utils import run_bass_kernel_spmd

    if "nc" not in _CACHE:
        _CACHE["nc"] = _build_nc()
    nc = _CACHE["nc"]

    w = _prep_weights(inputs)
    nodes = np.ascontiguousarray(np.asarray(inputs["encoded_nodes"], np.float32))
    q1 = np.ascontiguousarray(np.asarray(inputs["encoded_q1"], np.float32))
    last = np.ascontiguousarray(np.asarray(inputs["encoded_last_node"], np.float32))
    mask = np.ascontiguousarray(np.asarray(inputs["ninf_mask"], np.float32))

    in_maps = []
    for i in range(NCORES):
        sl = slice(i * BL, (i + 1) * BL)
        in_maps.append(
            {
                "nodes": nodes[sl],
                "q1": q1[sl],
                "last": last[sl],
                "mask": mask[sl],
                **w,
            }
        )
    try:
        res = run_bass_kernel_spmd(nc, in_maps, list(range(NCORES)), trace=trace)
    except Exception:
        # The first execution of a freshly compiled NEFF occasionally dies
        # with NRT_EXEC_UNIT_UNRECOVERABLE on this stack; a retry with the
        # cached NEFF has always succeeded.
        res = run_bass_kernel_spmd(nc, in_maps, list(range(NCORES)), trace=trace)
    out = np.concatenate(
        [res.results[i]["probs"].astype(np.float32) for i in range(NCORES)], axis=0
    )
    return out, res


def kernel(**inputs) -> np.ndarray:
    out, _ = run(inputs, trace=False)
    return out
